# revision 6
# baseline (speedup 1.0000x reference)
"""LSTM cell kernel for Trainium2, 8 NeuronCores, data-parallel over batch.

Math: stacked = x @ Wx + bx + prevh @ Wh
      i,f,o,g = split(stacked, 4, axis=1); i,f,o = sigmoid; g = tanh
      nextc = prevc*f + g*i ; nexth = tanh(nextc)*o

Device strategy (per core, batch shard of 1024 rows):
  - Host pre-concats [x|prevh] and [Wx;Wh] into one K=2048 contraction,
    pre-transposes activations to [K, B] bf16 (PE needs contraction on
    partitions), and reorders weight columns into per-gate 128-col blocks
    grouped by state block j: [i_j|f_j|o_j|g_j], so one 128-partition PSUM
    tile is exactly one gate for one state block.
  - matmul: stationary = W tile [k128, g128] (natural), moving = xhT
    [k128, b512]; PSUM out = [g128, b512] transposed layout. Bias becomes a
    per-partition ACT fused add; sigmoid/tanh fused into PSUM eviction.
  - Elementwise combine in [state, batch] layout; outputs written
    transposed and un-transposed on host.
"""

import sys

sys.path.insert(0, "/opt/trn_rl_repo")

import numpy as np

BATCH = 8192
DIM = 1024  # INPUT_DIM == STATE_DIM
K = 2 * DIM  # stacked contraction [x|prevh]
NCORES = 8
B_LOC = BATCH // NCORES  # 1024
N_KT = K // 128  # 16 k-tiles
N_GT = 4 * DIM // 128  # 32 gate-block tiles
N_J = DIM // 128  # 8 state blocks

_CACHED = {}


def _build_program():
    import ml_dtypes  # noqa: F401
    from concourse import bass, tile
    from concourse.bass import mybir

    bf16 = mybir.dt.bfloat16
    f32 = mybir.dt.float32
    AF = mybir.ActivationFunctionType

    nc = bass.Bass("TRN2", target_bir_lowering=False)
    xhT_d = nc.dram_tensor("xhT", [K, B_LOC], bf16, kind="ExternalInput")
    w_d = nc.dram_tensor("w", [N_GT, 128, K], bf16, kind="ExternalInput")
    bias_d = nc.dram_tensor("bias", [128, N_GT], f32, kind="ExternalInput")
    pcT_d = nc.dram_tensor("pcT", [DIM, B_LOC], f32, kind="ExternalInput")
    hT_d = nc.dram_tensor("hT", [DIM, B_LOC], f32, kind="ExternalOutput")
    cT_d = nc.dram_tensor("cT", [DIM, B_LOC], f32, kind="ExternalOutput")

    with tile.TileContext(nc) as tc:
        with (
            tc.tile_pool(name="const", bufs=1) as const_pool,
            tc.tile_pool(name="wp", bufs=4) as w_pool,
            tc.tile_pool(name="pc", bufs=3) as pc_pool,
            tc.tile_pool(name="gates", bufs=10) as g_pool,
            tc.tile_pool(name="outs", bufs=3) as out_pool,
            tc.tile_pool(name="psum", bufs=8, space="PSUM") as psum_pool,
        ):
            # resident activations: [128, kt*1024 + b] bf16, 32KB/partition
            xh_sb = const_pool.tile([128, N_KT * B_LOC], bf16)
            for kt in range(N_KT):
                nc.sync.dma_start(
                    xh_sb[:, kt * B_LOC : (kt + 1) * B_LOC],
                    xhT_d[kt * 128 : (kt + 1) * 128, :],
                )
            bias_sb = const_pool.tile([128, N_GT], f32)
            nc.sync.dma_start(bias_sb[:], bias_d[:])

            for j in range(N_J):
                pc_sb = pc_pool.tile([128, B_LOC], f32)
                nc.sync.dma_start(pc_sb[:], pcT_d[j * 128 : (j + 1) * 128, :])

                gs = []
                for gate in range(4):
                    gt = j * 4 + gate
                    w_sb = w_pool.tile([128, K], bf16)
                    nc.sync.dma_start(w_sb[:], w_d[gt])
                    ps0 = psum_pool.tile([128, 512], f32, tag="ps")
                    ps1 = psum_pool.tile([128, 512], f32, tag="ps")
                    for kt in range(N_KT):
                        lhsT = w_sb[:, kt * 128 : (kt + 1) * 128]
                        first, last = kt == 0, kt == N_KT - 1
                        nc.tensor.matmul(
                            ps0[:],
                            lhsT,
                            xh_sb[:, kt * B_LOC : kt * B_LOC + 512],
                            start=first,
                            stop=last,
                        )
                        nc.tensor.matmul(
                            ps1[:],
                            lhsT,
                            xh_sb[:, kt * B_LOC + 512 : (kt + 1) * B_LOC],
                            start=first,
                            stop=last,
                        )
                    g_sb = g_pool.tile([128, B_LOC], f32, tag="g")
                    func = AF.Tanh if gate == 3 else AF.Sigmoid
                    nc.scalar.activation(
                        g_sb[:, 0:512], ps0[:], func, bias=bias_sb[:, gt : gt + 1]
                    )
                    nc.scalar.activation(
                        g_sb[:, 512:B_LOC], ps1[:], func, bias=bias_sb[:, gt : gt + 1]
                    )
                    gs.append(g_sb)

                i_t, f_t, o_t, g_t = gs
                c_sb = out_pool.tile([128, B_LOC], f32, tag="c")
                tmp = out_pool.tile([128, B_LOC], f32, tag="tmp")
                nc.vector.tensor_mul(out=tmp[:], in0=i_t[:], in1=g_t[:])
                nc.vector.tensor_mul(out=c_sb[:], in0=f_t[:], in1=pc_sb[:])
                nc.vector.tensor_add(out=c_sb[:], in0=c_sb[:], in1=tmp[:])
                h_sb = out_pool.tile([128, B_LOC], f32, tag="h")
                nc.scalar.activation(h_sb[:], c_sb[:], AF.Tanh)
                nc.vector.tensor_mul(out=h_sb[:], in0=h_sb[:], in1=o_t[:])
                nc.sync.dma_start(cT_d[j * 128 : (j + 1) * 128, :], c_sb[:])
                nc.sync.dma_start(hT_d[j * 128 : (j + 1) * 128, :], h_sb[:])

    nc.finalize()
    _install_wait_splitter(nc)
    return nc


def _split_multiwaits(mod: dict) -> dict:
    """This container's walrus encodes at most ONE sync wait per instruction
    (setupSyncWait raises 'Too many sync wait commands'), while Tile emits
    several. Move excess waits onto standalone single-wait EventSemaphore
    instructions inserted just before, on the same engine. All excess waits
    must be monotone (sem-ge-imm) for the serialization to be equivalent.
    """
    for fn in mod.get("functions", []):
        for blk in fn.get("blocks", []):
            insts = blk.get("instructions") or []
            out = []
            for inst in insts:
                si = inst.get("sync_info")
                waits = (si or {}).get("on_wait") or []
                if len(waits) > 1:
                    keep, extra = [], []
                    # keep non-monotone waits (if any) on the instruction
                    for w in waits:
                        (extra if w.get("wait_mode") == "sem-ge-imm" else keep).append(w)
                    if not keep:
                        keep.append(extra.pop())
                    for n, w in enumerate(extra):
                        out.append(
                            {
                                "name": f"{inst['name']}_sw{n}",
                                "opcode": "EventSemaphore",
                                "engine": inst["engine"],
                                "debug": inst.get("debug", 0),
                                "sync_info": {"on_wait": [w], "on_update": []},
                            }
                        )
                    si["on_wait"] = keep
                out.append(inst)
            blk["instructions"] = out
    return mod


def _install_wait_splitter(nc):
    import json as _json

    orig = nc.to_json_bytes

    def patched():
        mod = _json.loads(orig())
        return _json.dumps(_split_multiwaits(mod)).encode()

    nc.to_json_bytes = patched


def _prep_shared(Wx, bx, Wh):
    import ml_dtypes

    bf16 = ml_dtypes.bfloat16
    W = np.concatenate([Wx, Wh], axis=0)  # [K, 4*DIM]
    # columns gate*DIM + j*128 + c  ->  (j*4 + gate)*128 + c
    W_re = W.reshape(K, 4, N_J, 128).transpose(0, 2, 1, 3).reshape(K, 4 * DIM)
    # device layout [gt, p(k%128), kt*128 + c]
    W_dev = np.ascontiguousarray(
        W_re.reshape(N_KT, 128, N_GT, 128).transpose(2, 1, 0, 3).reshape(N_GT, 128, K),
        dtype=bf16,
    )
    b_re = bx.reshape(4, N_J, 128).transpose(1, 0, 2).reshape(4 * DIM)
    bias_dev = np.ascontiguousarray(b_re.reshape(N_GT, 128).T, dtype=np.float32)
    return W_dev, bias_dev


def kernel(x, prevh, prevc, Wx, bx, Wh):
    import ml_dtypes
    from concourse import bass_utils

    bf16 = ml_dtypes.bfloat16

    if "nc" not in _CACHED:
        _CACHED["nc"] = _build_program()
    nc = _CACHED["nc"]

    W_dev, bias_dev = _prep_shared(Wx, bx, Wh)

    in_maps = []
    for c in range(NCORES):
        rows = slice(c * B_LOC, (c + 1) * B_LOC)
        xh = np.concatenate([x[rows], prevh[rows]], axis=1)  # [B_LOC, K]
        xhT = np.ascontiguousarray(xh.T, dtype=bf16)  # [K, B_LOC]
        pcT = np.ascontiguousarray(prevc[rows].T, dtype=np.float32)
        in_maps.append({"xhT": xhT, "w": W_dev, "bias": bias_dev, "pcT": pcT})
    _CACHED["in_maps"] = in_maps

    res = bass_utils.run_bass_kernel_spmd(nc, in_maps, core_ids=list(range(NCORES)))

    nexth = np.empty((BATCH, DIM), np.float32)
    nextc = np.empty((BATCH, DIM), np.float32)
    for c in range(NCORES):
        rows = slice(c * B_LOC, (c + 1) * B_LOC)
        nexth[rows] = np.asarray(res.results[c]["hT"]).T
        nextc[rows] = np.asarray(res.results[c]["cT"]).T
    return nexth, nextc


if __name__ == "__main__":
    rng = np.random.default_rng(0)
    inputs = {
        "x": rng.standard_normal((BATCH, DIM), np.float32),
        "prevh": rng.standard_normal((BATCH, DIM), np.float32),
        "prevc": rng.standard_normal((BATCH, DIM), np.float32),
        "Wx": (rng.random((DIM, 4 * DIM), np.float32) - 0.5) / 16,
        "bx": (rng.random(4 * DIM, np.float32) - 0.5) / 16,
        "Wh": (rng.random((DIM, 4 * DIM), np.float32) - 0.5) / 16,
    }
    h, c = kernel(**inputs)
    print("ok", h.shape, c.shape, h.dtype)


# revision 23
# speedup vs baseline: 31.7441x; 31.7441x over previous
"""LSTM cell kernel for Trainium2, 8 NeuronCores, data-parallel over batch.

Math: stacked = x @ Wx + bx + prevh @ Wh
      i,f,o,g = split(stacked, 4, axis=1); i,f,o = sigmoid; g = tanh
      nextc = prevc*f + g*i ; nexth = tanh(nextc)*o

Device strategy (per core, batch shard of 1024 rows):
  - Host pre-concats [x|prevh] and [Wx;Wh] into one K=2048 contraction,
    pre-transposes activations to [K, B] bf16 (PE needs contraction on
    partitions), and reorders weight columns into per-gate 128-col blocks
    grouped by state block j: [i_j|f_j|o_j|g_j], so one 128-partition PSUM
    tile is exactly one gate for one state block.
  - matmul: stationary = W tile [k128, g128] (natural), moving = xhT
    [k128, b512]; PSUM out = [g128, b512] transposed layout. Bias becomes a
    per-partition ACT fused add; sigmoid/tanh fused into PSUM eviction.
  - Elementwise combine in [state, batch] layout; outputs written
    transposed and un-transposed on host.
"""

import os
import sys

sys.path.insert(0, "/opt/trn_rl_repo")
# v2 ASAP tile scheduler: measurably tighter schedule than the legacy flow
os.environ.setdefault("TILE_SCHEDULER", "asap")

import numpy as np

BATCH = 8192
DIM = 1024  # INPUT_DIM == STATE_DIM
K = 2 * DIM  # stacked contraction [x|prevh]
NCORES = 8
B_LOC = BATCH // NCORES  # 1024
N_KT = K // 128  # 16 k-tiles
N_GT = 4 * DIM // 128  # 32 gate-block tiles
N_J = DIM // 128  # 8 state blocks

_CACHED = {}


def _build_program():
    import ml_dtypes  # noqa: F401
    from concourse import bass, tile
    from concourse.bass import mybir

    bf16 = mybir.dt.bfloat16
    f32 = mybir.dt.float32
    AF = mybir.ActivationFunctionType

    nc = bass.Bass("TRN2", target_bir_lowering=False)
    xhT_d = nc.dram_tensor("xhT", [K, B_LOC], bf16, kind="ExternalInput")
    w_d = nc.dram_tensor("w", [N_GT, 128, K], bf16, kind="ExternalInput")
    bias_d = nc.dram_tensor("bias", [128, N_GT], f32, kind="ExternalInput")
    pcT_d = nc.dram_tensor("pcT", [DIM, B_LOC], f32, kind="ExternalInput")
    hT_d = nc.dram_tensor("hT", [DIM, B_LOC], f32, kind="ExternalOutput")
    cT_d = nc.dram_tensor("cT", [DIM, B_LOC], f32, kind="ExternalOutput")

    with tile.TileContext(nc) as tc:
        with (
            tc.tile_pool(name="const", bufs=1) as const_pool,
            tc.tile_pool(name="wp", bufs=6) as w_pool,
            tc.tile_pool(name="pc", bufs=3) as pc_pool,
            tc.tile_pool(name="gates", bufs=10) as g_pool,
            tc.tile_pool(name="outs", bufs=3) as out_pool,
            tc.tile_pool(name="psum", bufs=8, space="PSUM") as psum_pool,
        ):
            # resident activations: [128, kt*1024 + b] bf16, 32KB/partition.
            # Device gate order within each state block j is (i, f, g, o) —
            # host reorders W/bias to match — so everything except
            # sigmoid(o)*tanh(c) completes during o's matmuls.
            xh_sb = const_pool.tile([128, N_KT * B_LOC], bf16)
            bias_sb = const_pool.tile([128, N_GT], f32)

            def load_xh(kt):
                nc.sync.dma_start(
                    xh_sb[:, kt * B_LOC : (kt + 1) * B_LOC],
                    xhT_d[kt * 128 : (kt + 1) * 128, :],
                )

            w_tiles = {}

            def load_w(gt):
                w_sb = w_pool.tile([128, K], bf16, tag="w")
                nc.sync.dma_start(w_sb[:], w_d[gt])
                w_tiles[gt] = w_sb

            # interleave the startup DMAs: first w tiles between xh tiles so
            # PE can start on (w0, xh0) almost immediately
            load_w(0)
            load_xh(0)
            load_w(1)
            nc.scalar.dma_start(bias_sb[:], bias_d[:])
            for kt in range(1, N_KT):
                load_xh(kt)
            load_w(2)
            load_w(3)

            # dummy matmuls while the startup DMAs stream: warms the PE HAM
            # clock gate (3.4us busy window) so real matmuls run at 2.4GHz
            warm_sb = const_pool.tile([1, 128], bf16)
            nc.gpsimd.memset(warm_sb[:], 0.0)
            warm_ps = psum_pool.tile([128, 512], f32, tag="ps")
            for _ in range(40):
                nc.tensor.matmul(
                    warm_ps[:, 0:64],
                    warm_sb[:, 0:128],
                    warm_sb[:, 0:64],
                    start=True,
                    stop=True,
                )

            for j in range(N_J):
                pc_sb = pc_pool.tile([128, B_LOC], f32)
                nc.scalar.dma_start(pc_sb[:], pcT_d[j * 128 : (j + 1) * 128, :])

                def run_gates(gates, evict=True):
                    """Issue matmuls for one or more gates, kt-interleaved
                    when len>1 (keeps PE fed while xh tiles still stream in
                    during j=0)."""
                    tiles = {}
                    for gate in gates:
                        gt = j * 4 + gate
                        if gt not in w_tiles:
                            load_w(gt)
                        ps0 = psum_pool.tile([128, 512], f32, tag="ps")
                        ps1 = psum_pool.tile([128, 512], f32, tag="ps")
                        tiles[gate] = (w_tiles.pop(gt), ps0, ps1)
                    for kt in range(N_KT):
                        first, last = kt == 0, kt == N_KT - 1
                        for gate in gates:
                            w_sb, ps0, ps1 = tiles[gate]
                            lhsT = w_sb[:, kt * 128 : (kt + 1) * 128]
                            nc.tensor.matmul(
                                ps0[:],
                                lhsT,
                                xh_sb[:, kt * B_LOC : kt * B_LOC + 512],
                                start=first,
                                stop=last,
                            )
                            nc.tensor.matmul(
                                ps1[:],
                                lhsT,
                                xh_sb[:, kt * B_LOC + 512 : (kt + 1) * B_LOC],
                                start=first,
                                stop=last,
                            )
                    if not evict:
                        return tiles
                    for gate in gates:
                        gt = j * 4 + gate
                        _, ps0, ps1 = tiles[gate]
                        g_sb = g_pool.tile([128, B_LOC], f32, tag="g")
                        func = AF.Tanh if gate == 2 else AF.Sigmoid
                        nc.scalar.activation(
                            g_sb[:, 0:512], ps0[:], func, bias=bias_sb[:, gt : gt + 1]
                        )
                        nc.scalar.activation(
                            g_sb[:, 512:B_LOC],
                            ps1[:],
                            func,
                            bias=bias_sb[:, gt : gt + 1],
                        )
                        gs.append(g_sb)
                    return tiles

                gs = []
                last_j = j == N_J - 1
                gate_groups = [(0, 1), (2,)] if j == 0 else [(0,), (1,), (2,)]
                for gates in gate_groups:
                    run_gates(gates)
                    if len(gs) == 3:
                        # i, f, g ready: compute c and tanh(c) while o's
                        # matmuls run
                        i_t, f_t, g_t = gs
                        c_sb = out_pool.tile([128, B_LOC], f32, tag="c")
                        tmp = out_pool.tile([128, B_LOC], f32, tag="tmp")
                        nc.vector.tensor_mul(out=tmp[:], in0=i_t[:], in1=g_t[:])
                        nc.vector.tensor_mul(out=c_sb[:], in0=f_t[:], in1=pc_sb[:])
                        nc.vector.tensor_add(out=c_sb[:], in0=c_sb[:], in1=tmp[:])
                        nc.scalar.dma_start(
                            cT_d[j * 128 : (j + 1) * 128, :], c_sb[:]
                        )
                        h_sb = out_pool.tile([128, B_LOC], f32, tag="h")
                        nc.scalar.activation(h_sb[:], c_sb[:], AF.Tanh)

                if not last_j:
                    run_gates((3,))
                    o_t = gs[3]
                    nc.vector.tensor_mul(out=h_sb[:], in0=h_sb[:], in1=o_t[:])
                    nc.scalar.dma_start(hT_d[j * 128 : (j + 1) * 128, :], h_sb[:])
                else:
                    # final state block: chunk the o-gate epilogue so the
                    # post-last-matmul serial chain is ~256 cols, not 1024
                    tiles = run_gates((3,), evict=False)
                    _, ps0, ps1 = tiles[3]
                    gt = j * 4 + 3
                    o_sb = g_pool.tile([128, B_LOC], f32, tag="g")
                    for cb in range(4):
                        ps = ps0 if cb < 2 else ps1
                        pslice = slice((cb % 2) * 256, (cb % 2) * 256 + 256)
                        bslice = slice(cb * 256, (cb + 1) * 256)
                        nc.scalar.activation(
                            o_sb[:, bslice],
                            ps[:, pslice],
                            AF.Sigmoid,
                            bias=bias_sb[:, gt : gt + 1],
                        )
                        nc.vector.tensor_mul(
                            out=o_sb[:, bslice],
                            in0=o_sb[:, bslice],
                            in1=h_sb[:, bslice],
                        )
                        nc.sync.dma_start(
                            hT_d[j * 128 : (j + 1) * 128, bslice], o_sb[:, bslice]
                        )

    nc.finalize()
    _install_wait_splitter(nc)
    return nc


def _split_multiwaits(mod: dict) -> dict:
    """This container's walrus encodes at most ONE sync wait per instruction
    (setupSyncWait raises 'Too many sync wait commands'), while Tile emits
    several. Move excess waits onto standalone single-wait EventSemaphore
    instructions inserted just before, on the same engine. All excess waits
    must be monotone (sem-ge-imm) for the serialization to be equivalent.
    """
    for fn in mod.get("functions", []):
        for blk in fn.get("blocks", []):
            insts = blk.get("instructions") or []
            out = []
            for inst in insts:
                si = inst.get("sync_info")
                waits = (si or {}).get("on_wait") or []
                if len(waits) > 1:
                    keep, extra = [], []
                    # keep non-monotone waits (if any) on the instruction
                    for w in waits:
                        (extra if w.get("wait_mode") == "sem-ge-imm" else keep).append(w)
                    if not keep:
                        keep.append(extra.pop())
                    for n, w in enumerate(extra):
                        out.append(
                            {
                                "name": f"{inst['name']}_sw{n}",
                                "opcode": "EventSemaphore",
                                "engine": inst["engine"],
                                "debug": inst.get("debug", 0),
                                "sync_info": {"on_wait": [w], "on_update": []},
                            }
                        )
                    si["on_wait"] = keep
                out.append(inst)
            blk["instructions"] = out
    return mod


def _install_wait_splitter(nc):
    import json as _json

    orig = nc.to_json_bytes

    def patched():
        mod = _json.loads(orig())
        return _json.dumps(_split_multiwaits(mod)).encode()

    nc.to_json_bytes = patched


def _prep_shared(Wx, bx, Wh):
    import ml_dtypes

    bf16 = ml_dtypes.bfloat16
    W = np.concatenate([Wx, Wh], axis=0)  # [K, 4*DIM]
    # columns gate*DIM + j*128 + c -> (j*4 + pos)*128 + c with device gate
    # order (i, f, g, o) within each state block j
    W_re = (
        W.reshape(K, 4, N_J, 128)[:, [0, 1, 3, 2]]
        .transpose(0, 2, 1, 3)
        .reshape(K, 4 * DIM)
    )
    # device layout [gt, p(k%128), kt*128 + c]
    W_dev = np.ascontiguousarray(
        W_re.reshape(N_KT, 128, N_GT, 128).transpose(2, 1, 0, 3).reshape(N_GT, 128, K),
        dtype=bf16,
    )
    b_re = bx.reshape(4, N_J, 128)[[0, 1, 3, 2]].transpose(1, 0, 2).reshape(4 * DIM)
    bias_dev = np.ascontiguousarray(b_re.reshape(N_GT, 128).T, dtype=np.float32)
    return W_dev, bias_dev


def kernel(x, prevh, prevc, Wx, bx, Wh):
    import ml_dtypes
    from concourse import bass_utils

    bf16 = ml_dtypes.bfloat16

    if "nc" not in _CACHED:
        _CACHED["nc"] = _build_program()
    nc = _CACHED["nc"]

    W_dev, bias_dev = _prep_shared(Wx, bx, Wh)

    in_maps = []
    for c in range(NCORES):
        rows = slice(c * B_LOC, (c + 1) * B_LOC)
        xh = np.concatenate([x[rows], prevh[rows]], axis=1)  # [B_LOC, K]
        xhT = np.ascontiguousarray(xh.T, dtype=bf16)  # [K, B_LOC]
        pcT = np.ascontiguousarray(prevc[rows].T, dtype=np.float32)
        in_maps.append({"xhT": xhT, "w": W_dev, "bias": bias_dev, "pcT": pcT})
    _CACHED["in_maps"] = in_maps

    res = bass_utils.run_bass_kernel_spmd(nc, in_maps, core_ids=list(range(NCORES)))

    nexth = np.empty((BATCH, DIM), np.float32)
    nextc = np.empty((BATCH, DIM), np.float32)
    for c in range(NCORES):
        rows = slice(c * B_LOC, (c + 1) * B_LOC)
        nexth[rows] = np.asarray(res.results[c]["hT"]).T
        nextc[rows] = np.asarray(res.results[c]["cT"]).T
    return nexth, nextc


if __name__ == "__main__":
    rng = np.random.default_rng(0)
    inputs = {
        "x": rng.standard_normal((BATCH, DIM), np.float32),
        "prevh": rng.standard_normal((BATCH, DIM), np.float32),
        "prevc": rng.standard_normal((BATCH, DIM), np.float32),
        "Wx": (rng.random((DIM, 4 * DIM), np.float32) - 0.5) / 16,
        "bx": (rng.random(4 * DIM, np.float32) - 0.5) / 16,
        "Wh": (rng.random((DIM, 4 * DIM), np.float32) - 0.5) / 16,
    }
    h, c = kernel(**inputs)
    print("ok", h.shape, c.shape, h.dtype)
